# revision 1
# baseline (speedup 1.0000x reference)
"""Trainium2 Bass kernel for the gated dual-softmax attention problem.

Shapes (hardcoded): x [4,1024,256], pos [4,1024,16], H=8 heads, dh=32.

Math notes (exact reformulations of the reference):
  * pos_logits[b,h,i,j] = (p[b,i]-p[b,j])@Wh[:,h] + bh[h].  Under softmax
    over j the i-dependent terms are constants, so
    pos_attn[b,h,i,j] = softmax_j(-p[b,j]@Wh[:,h]) =: w[b,h,j]  (no i dep).
    Its contribution to the output is the single vector w @ v_h.
  * Both softmaxes sum to 1, so the renormalization is an exact no-op.
  * Scores are O(+-8) for these inputs, so exp() without max-subtraction is
    safe; the normalization divides it out exactly.

Sharding: 8 cores = 4 batches x 2 query-halves. Each core computes the
full attention for 512 query rows of one batch (keys/values over all 1024
rows) including the output projection - no cross-core math, host only
concatenates the 8 [512,256] slices.

Layout: everything transposed so softmax reductions are free-axis and the
attn@v matmul needs no transposes:
  scoresT[j,i] per (head, key-chunk) via lhsT=kT chunk, rhs=qT
  eT = exp(scoresT/sqrt(dh))  (ACT, psum->sbuf)
  oT'[d,i] accumulates over key chunks with lhsT=v_aug [128,33] where
  col 32 = 1/(1-g_h): row 32 of oT' is then S_i/(1-g_h); its reciprocal
  times oT' rows 0..31 gives (1-g_h)*attn@v directly.
  The +g_h*(w@v_h) term is folded into an effective bias:
  bo_eff = bo + sum_h g_h*(w_h@v_h)@Wo_h, added after the Wo projection.
"""

import sys

if "/opt/trn_rl_repo" not in sys.path:
    sys.path.insert(0, "/opt/trn_rl_repo")

import numpy as np

B, N, D, H, DH, DP, PD = 4, 1024, 256, 8, 32, 32, 16
NQ = N // 2          # query rows per core
NCORES = 8
INV_C = 1.0 / np.sqrt(DH)

_nc_cache = {}


def _build_nc(reps=1, abl=4):
    from contextlib import ExitStack

    import concourse.bass as bass
    import concourse.tile as tile
    from concourse import bacc, mybir

    f32 = mybir.dt.float32
    f32r = mybir.dt.float32r
    AL = mybir.AluOpType

    nc = bacc.Bacc("TRN2", target_bir_lowering=False, debug=False,
                   num_devices=NCORES)

    din = {}
    for name, shape in [
        ("xqT", [D, NQ]), ("xkvT", [D, N]), ("posT", [PD, N]),
        ("Wq", [D, D]), ("Wk", [D, D]), ("Wv", [D, D]), ("Wo", [D, D]),
        ("Wp1", [PD, PD]), ("bp1", [PD, 1]), ("Wp2", [PD, DP]),
        ("Wh", [DP, H]), ("gvec", [H, 1]), ("cinv", [1, H]), ("bo", [1, D]),
    ]:
        din[name] = nc.dram_tensor(name, shape, f32, kind="ExternalInput").ap()
    dout = nc.dram_tensor("out", [NQ, D], f32, kind="ExternalOutput").ap()
    # DRAM scratch for partition-broadcasts (SBUF APs can't have step-0
    # partition dims, so broadcasts bounce through DRAM)
    dscr_r = nc.dram_tensor("scr_r", [H, NQ], f32, kind="Internal").ap()
    dscr_b = nc.dram_tensor("scr_b", [1, D], f32, kind="Internal").ap()

    with tile.TileContext(nc) as tc, ExitStack() as ctx:
        raw = ctx.enter_context(tc.tile_pool(name="raw", bufs=1))
        persist = ctx.enter_context(tc.tile_pool(name="persist", bufs=1))
        et_pool = ctx.enter_context(tc.tile_pool(name="et", bufs=10))
        small = ctx.enter_context(tc.tile_pool(name="small", bufs=1))
        outp = ctx.enter_context(tc.tile_pool(name="outp", bufs=2))
        # PSUM: sc 2x2 banks + work 2x1 + oacc 2x1 = 8 banks
        ps_sc = ctx.enter_context(tc.tile_pool(name="ps_sc", bufs=1, space="PSUM"))
        ps_wk = ctx.enter_context(tc.tile_pool(name="ps_wk", bufs=2, space="PSUM"))
        ps_oa = ctx.enter_context(tc.tile_pool(name="ps_oa", bufs=2, space="PSUM"))

        def load_round(ap_dram, shape, tag):
            """DMA a DRAM tensor to SBUF (chunked across queues) and round
            it to fp32r via DVE."""
            t0 = raw.tile(shape, f32, tag=tag)
            cols = shape[-1]
            nch = max(1, cols // 256) if len(shape) == 2 and shape[0] >= 128 \
                else 1
            for c in range(nch):
                sl = slice(c * cols // nch, (c + 1) * cols // nch)
                nc.sync.dma_start(out=t0[:, sl], in_=ap_dram[:, sl])
            t1 = persist.tile(shape, f32r, tag=tag + "_r")
            nc.vector.tensor_copy(out=t1, in_=t0)
            return t1

        # ---- input loads + fp32r rounding ----
        xq = [load_round(din["xqT"][c * 128:(c + 1) * 128, :], [128, NQ],
                         f"xq{c}") for c in range(2)]
        xkv = [load_round(din["xkvT"][c * 128:(c + 1) * 128, :], [128, N],
                          f"xkv{c}") for c in range(2)]
        wq = [load_round(din["Wq"][c * 128:(c + 1) * 128, :], [128, D],
                         f"wq{c}") for c in range(2)]
        wk = [load_round(din["Wk"][c * 128:(c + 1) * 128, :], [128, D],
                         f"wk{c}") for c in range(2)]
        wv = [load_round(din["Wv"][c * 128:(c + 1) * 128, :], [128, D],
                         f"wv{c}") for c in range(2)]
        wo = [load_round(din["Wo"][c * 128:(c + 1) * 128, :], [128, D],
                         f"wo{c}") for c in range(2)]
        posr = load_round(din["posT"], [PD, N], "posr")
        wp1 = load_round(din["Wp1"], [PD, PD], "wp1")
        wp2 = load_round(din["Wp2"], [PD, DP], "wp2")
        wh = load_round(din["Wh"], [DP, H], "wh")

        bp1 = persist.tile([PD, 1], f32)
        nc.sync.dma_start(out=bp1, in_=din["bp1"])
        gv = persist.tile([H, 1], f32)
        nc.sync.dma_start(out=gv, in_=din["gvec"])
        bo_sb = persist.tile([1, D], f32)
        nc.sync.dma_start(out=bo_sb, in_=din["bo"])
        # cinv broadcast to all partitions (becomes col 32 of v_aug tiles)
        cbc = persist.tile([128, H], f32)
        cin = din["cinv"]
        nc.sync.dma_start(
            out=cbc, in_=bass.AP(tensor=cin.tensor, offset=cin.offset,
                                 ap=[[0, 128]] + cin.ap[1:]))
        ident = persist.tile([PD, PD], f32)
        from concourse.masks import make_identity
        make_identity(nc, ident[:])
        ones_f = persist.tile([1, DH], f32)
        nc.vector.memset(ones_f, 1.0)
        ones_r = persist.tile([1, DH], f32r)
        nc.vector.tensor_copy(out=ones_r, in_=ones_f)

        def body():
            # ---- projections ----
            # qT_all [256, 512]: row d = (x[rows] @ W)[:, d]
            qT = []
            for mc in range(2):
                p = ps_wk.tile([128, NQ], f32, tag="wk")
                for kc in range(2):
                    nc.tensor.matmul(
                        p, lhsT=wq[kc][:, mc * 128:(mc + 1) * 128],
                        rhs=xq[kc], start=(kc == 0), stop=(kc == 1))
                t = persist.tile([128, NQ], f32r, tag=f"qT{mc}")
                nc.vector.tensor_copy(out=t, in_=p)
                qT.append(t)
            # kT_all [256, 1024]
            kT = []
            for mc in range(2):
                t = persist.tile([128, N], f32r, tag=f"kT{mc}")
                for nn in range(2):
                    p = ps_wk.tile([128, NQ], f32, tag="wk")
                    for kc in range(2):
                        nc.tensor.matmul(
                            p, lhsT=wk[kc][:, mc * 128:(mc + 1) * 128],
                            rhs=xkv[kc][:, nn * NQ:(nn + 1) * NQ],
                            start=(kc == 0), stop=(kc == 1))
                    nc.vector.tensor_copy(out=t[:, nn * NQ:(nn + 1) * NQ], in_=p)
                kT.append(t)
            # v in row layout, augmented: v_sb[rc] is [128, H, DH+1], col DH = cinv
            v_sb = []
            v_pure = []
            for rc in range(8):
                p = ps_wk.tile([128, D], f32, tag="wk")
                for kc in range(2):
                    nc.tensor.matmul(
                        p, lhsT=xkv[kc][:, rc * 128:(rc + 1) * 128], rhs=wv[kc],
                        start=(kc == 0), stop=(kc == 1))
                t = persist.tile([128, H, DH + 1], f32r, tag=f"v{rc}")
                nc.vector.tensor_copy(
                    out=t[:, :, 0:DH],
                    in_=p.rearrange("p (h d) -> p h d", h=H))
                nc.vector.tensor_copy(out=t[:, :, DH], in_=cbc)
                v_sb.append(t)
                tp = persist.tile([128, D], f32r, tag=f"vp{rc}", name=f"vp{rc}")
                nc.vector.tensor_copy(out=tp, in_=p)
                v_pure.append(tp)

            # ---- pos branch (collapsed to O(N)) ----
            h1 = small.tile([PD, N], f32r, tag="h1")
            for nn in range(2):
                h1p = ps_wk.tile([PD, NQ], f32, tag="wk", name="h1p")
                nc.tensor.matmul(h1p, lhsT=wp1,
                                 rhs=posr[:, nn * NQ:(nn + 1) * NQ],
                                 start=True, stop=True)
                nc.vector.tensor_scalar(out=h1[:, nn * NQ:(nn + 1) * NQ],
                                        in0=h1p, scalar1=bp1, scalar2=0.0,
                                        op0=AL.add, op1=AL.max)
            p_sb = small.tile([DP, N], f32r, tag="p_sb")
            for nn in range(2):
                pp = ps_wk.tile([DP, NQ], f32, tag="wk", name="pp")
                nc.tensor.matmul(pp, lhsT=wp2,
                                 rhs=h1[:, nn * NQ:(nn + 1) * NQ],
                                 start=True, stop=True)
                nc.vector.tensor_copy(out=p_sb[:, nn * NQ:(nn + 1) * NQ],
                                      in_=pp)
            ep = small.tile([H, N], f32, tag="ep")
            mnh = small.tile([H, 2], f32, tag="mnh")
            sp_halves = []
            for nn in range(2):
                sp = ps_wk.tile([H, NQ], f32, tag="wk", name="sp")
                nc.tensor.matmul(sp, lhsT=wh,
                                 rhs=p_sb[:, nn * NQ:(nn + 1) * NQ],
                                 start=True, stop=True)
                nc.vector.tensor_reduce(out=mnh[:, nn:nn + 1], in_=sp,
                                        axis=mybir.AxisListType.X, op=AL.min)
                sp_halves.append(sp)
            mn = small.tile([H, 1], f32, tag="mn")
            nc.vector.tensor_reduce(out=mn, in_=mnh,
                                    axis=mybir.AxisListType.X, op=AL.min)
            for nn in range(2):
                nc.scalar.activation(out=ep[:, nn * NQ:(nn + 1) * NQ],
                                     in_=sp_halves[nn],
                                     func=mybir.ActivationFunctionType.Exp,
                                     scale=-1.0, bias=mn)
            sp_sum = small.tile([H, 1], f32, tag="sp_sum")
            nc.vector.tensor_reduce(out=sp_sum, in_=ep,
                                    axis=mybir.AxisListType.X, op=AL.add)
            rp = small.tile([H, 1], f32, tag="rp")
            nc.vector.reciprocal(out=rp, in_=sp_sum)
            gr = small.tile([H, 1], f32, tag="gr")
            nc.vector.tensor_mul(gr, gv, rp)
            eps = small.tile([H, N], f32, tag="eps")
            nc.vector.tensor_scalar_mul(eps, ep, gr)  # g_h * pos_attn row
            # transpose to [N, H] in 8 chunks of 128
            epT = []
            for jc in range(8):
                ptr = ps_wk.tile([128, H], f32, tag="wk")
                nc.tensor.transpose(ptr[:, 0:H],
                                    eps[:, jc * 128:(jc + 1) * 128], ident[0:H, 0:H])
                t = small.tile([128, H], f32r, tag=f"epT{jc}")
                nc.vector.tensor_copy(out=t, in_=ptr[:, 0:H])
                epT.append(t)
            # PVmat [256, H] = sum_j v[j, :] * (g*w)[j, h]
            pvg = []
            for mc in range(2):
                p = ps_wk.tile([128, H], f32, tag="wk")
                for jc in range(8):
                    nc.tensor.matmul(
                        p, lhsT=v_pure[jc][:, mc * 128:(mc + 1) * 128],
                        rhs=epT[jc], start=(jc == 0), stop=(jc == 7))
                t = small.tile([128, 1], f32r, tag=f"pvg{mc}")
                for hh in range(4):
                    h = mc * 4 + hh
                    nc.vector.tensor_copy(out=t[hh * DH:(hh + 1) * DH, 0:1],
                                          in_=p[hh * DH:(hh + 1) * DH, h:h + 1])
                pvg.append(t)
            pwo = ps_wk.tile([1, D], f32, tag="wk")
            for mc in range(2):
                nc.tensor.matmul(pwo, lhsT=pvg[mc], rhs=wo[mc],
                                 start=(mc == 0), stop=(mc == 1))
            bo_eff = small.tile([1, D], f32, tag="bo_eff")
            nc.vector.tensor_add(bo_eff, bo_sb, pwo)
            bo_bc = persist.tile([128, D], f32)
            nc.sync.dma_start(out=dscr_b, in_=bo_eff)
            nc.sync.dma_start(
                out=bo_bc, in_=bass.AP(tensor=dscr_b.tensor, offset=dscr_b.offset,
                                       ap=[[0, 128], [1, D]]))

            # ---- main attention ----
            oT = [persist.tile([128, NQ], f32r, tag=f"oT{mc}", name=f"oT{mc}")
                  for mc in range(2)] if abl >= 3 else []
            for mc in range(2):
                if abl < 1:
                    break
                ets = []
                for kc in range(8):
                    scp = ps_sc.tile([128, 4, NQ], f32, tag="sc", name="scp")
                    for ht in range(4):
                        nc.tensor.matmul(
                            scp[:, ht, :],
                            lhsT=kT[mc][ht * DH:(ht + 1) * DH,
                                        kc * 128:(kc + 1) * 128],
                            rhs=qT[mc][ht * DH:(ht + 1) * DH, :],
                            start=True, stop=True,
                            tile_position=(ht * DH, 0))
                    if abl >= 2:
                        et = et_pool.tile([128, 4, NQ], f32r, tag="et",
                                          name="et")
                        nc.scalar.activation(
                            out=et, in_=scp,
                            func=mybir.ActivationFunctionType.Exp,
                            scale=INV_C)
                        ets.append(et)
                if abl < 3:
                    continue
                def epilogue(ht, op):
                    h = mc * 4 + ht
                    osb = outp.tile([DH + 1, NQ], f32, tag="osb_e",
                                    name="osb")
                    nc.vector.tensor_copy(out=osb, in_=op)
                    rr = outp.tile([1, NQ], f32r, tag="rr", name="rr")
                    with nc.allow_low_precision(
                            reason="fp32r recip feeds fp32r matmul"):
                        nc.vector.reciprocal(out=rr, in_=osb[DH:DH + 1, :])
                    rbp = ps_wk.tile([DH, NQ], f32, tag="wk", name="rbp")
                    nc.tensor.matmul(rbp, lhsT=ones_r, rhs=rr,
                                     start=True, stop=True)
                    nc.vector.tensor_mul(
                        oT[mc][ht * DH:(ht + 1) * DH, :],
                        osb[0:DH, :], rbp)

                for ht in (0, 1):
                    op = ps_oa.tile([DH + 1, NQ], f32, tag="oa", name="op")
                    for jc in range(8):
                        nc.tensor.matmul(op, lhsT=v_sb[jc][:, mc * 4 + ht, :],
                                         rhs=ets[jc][:, ht, :],
                                         start=(jc == 0), stop=(jc == 7))
                    epilogue(ht, op)
                # last pair jc-synced: only one matmul round after final exp
                op2 = ps_oa.tile([DH + 1, NQ], f32, tag="oa", name="op2a")
                op3 = ps_oa.tile([DH + 1, NQ], f32, tag="oa", name="op3a")
                for jc in range(8):
                    for s, opx in ((0, op2), (1, op3)):
                        nc.tensor.matmul(
                            opx, lhsT=v_sb[jc][:, mc * 4 + 2 + s, :],
                            rhs=ets[jc][:, 2 + s, :],
                            start=(jc == 0), stop=(jc == 7))
                epilogue(2, op2)
                epilogue(3, op3)

            # ---- output projection ----
            for qc in range(4):
                if abl < 3:
                    nc.sync.dma_start(out=dout[qc * 128:(qc + 1) * 128, :],
                                      in_=bo_bc)
                    continue
                p = ps_wk.tile([128, D], f32, tag="wk")
                for mc in range(2):
                    nc.tensor.matmul(p, lhsT=oT[mc][:, qc * 128:(qc + 1) * 128],
                                     rhs=wo[mc], start=(mc == 0), stop=(mc == 1))
                t = outp.tile([128, D], f32, tag="osb")
                nc.vector.tensor_add(t, p, bo_bc)
                nc.sync.dma_start(out=dout[qc * 128:(qc + 1) * 128, :], in_=t)

        if reps == 1:
            body()
        elif reps <= 4:
            for _ in range(reps):
                body()
        else:
            with tc.For_i(0, reps, 1):
                body()

    nc.compile()
    return nc


def _get_nc():
    if "nc" not in _nc_cache:
        _nc_cache["nc"] = _build_nc()
    return _nc_cache["nc"]


def kernel(**inputs):
    from concourse.bass_utils import run_bass_kernel_spmd

    x = np.ascontiguousarray(np.asarray(inputs["x"], dtype=np.float32))
    pos = np.ascontiguousarray(np.asarray(inputs["pos"], dtype=np.float32))
    W = {k: np.ascontiguousarray(np.asarray(inputs[k], dtype=np.float32))
         for k in ["Wq", "Wk", "Wv", "Wo", "Wp1", "Wp2", "Wh"]}
    bp1 = np.asarray(inputs["bp1"], np.float32).reshape(PD, 1)
    bo = np.asarray(inputs["bo"], np.float32).reshape(1, D)
    gate = np.asarray(inputs["gate"], np.float32)
    g = (1.0 / (1.0 + np.exp(-gate.astype(np.float64)))).astype(np.float32)
    cinv = (1.0 / (1.0 - g.astype(np.float64))).astype(np.float32)

    nc = _get_nc()
    in_maps = []
    for core in range(NCORES):
        b, half = divmod(core, 2)
        q0 = half * NQ
        in_maps.append({
            "xqT": np.ascontiguousarray(x[b, q0:q0 + NQ, :].T),
            "xkvT": np.ascontiguousarray(x[b].T),
            "posT": np.ascontiguousarray(pos[b].T),
            "Wq": W["Wq"], "Wk": W["Wk"], "Wv": W["Wv"], "Wo": W["Wo"],
            "Wp1": W["Wp1"], "bp1": bp1, "Wp2": W["Wp2"], "Wh": W["Wh"],
            "gvec": g.reshape(H, 1), "cinv": cinv.reshape(1, H), "bo": bo,
        })
    res = run_bass_kernel_spmd(nc, in_maps, core_ids=list(range(NCORES)))
    out = np.empty((B, N, D), np.float32)
    for core in range(NCORES):
        b, half = divmod(core, 2)
        out[b, half * NQ:(half + 1) * NQ, :] = res.results[core]["out"]
    return out

